# revision 9
# baseline (speedup 1.0000x reference)
"""Dense mean-field CRF (2-label Potts, gaussian + bilateral pairwise) on 8
Trainium2 NeuronCores.

Math: the bilateral kernel factorizes as S_spatial (separable, sigma=50) o
B_intensity (gaussian gram on pixel values). B ~= P @ P.T (Nystrom over 256
landmark intensities) at rank 31, so each mean-field message is 32 separable
96x96 convolution channels (31 bilateral + 1 gaussian with sigma=3):

    msg = sum_ch praw_ch o (R_ch (p10_ch o h) R_ch),  R = S or G per channel
    logit = cb + msg - 13*h,   h = tanh(logit/2)

(the self-exclusion and Potts row terms collapse into the -13h and cb.)

Distribution: fully replicated — every core computes the identical full
problem, so there are no collectives, no cross-core rendezvous, and the
measured span is single-core latency. Per iteration the engines split:
PE runs 64 per-rank 96x96x96 f32 matmuls (exact f32 mandatory: any reduced-
precision matmul mode flips argmax pixels after 5x error amplification),
ACT does the PSUM->SBUF stage copies + tanh, DVE does the two elementwise
muls and a contiguous tree reduction over channels. Iteration 1's p10 o h0
is a host constant (h0 = tanh(cb/2) depends only on the mask), so iteration
1 runs without the wp multiply while input DMAs stream in. A few dummy
matmuls spaced along the DVE tree keep the PE's HAM clock-gate warm across
the inter-iteration PE-idle window.
"""
import sys
sys.path.insert(0, '/opt/trn_rl_repo')
import numpy as np

H = W = 96
KRANK = 28               # bilateral Nystrom rank
CHUNKS = [8, 8, 8, 4]    # bilateral channels per chunk (gaussian separate)
NCHUNK = len(CHUNKS)
CW = 8 * 96              # full chunk width in sbuf cols = 768
KW = KRANK * 96          # 2688
NITER = 5
NCORES = 8
EPS = 1e-8

_CACHE = {}
LAST_RESULTS = None


# ------------------------- host precomputation -------------------------

def _nystrom_P(f64, krank=KRANK):
    """Rank-k factor P [N, k] with exp(-(fi-fj)^2/400) ~= P @ P.T"""
    t = np.linspace(f64.min() - 1.0, f64.max() + 1.0, 256)
    Ktt = np.exp(-(t[:, None] - t[None, :]) ** 2 / 400.0)
    Kft = np.exp(-(f64[:, None] - t[None, :]) ** 2 / 400.0)
    lam, V = np.linalg.eigh(Ktt)
    keep = lam > lam.max() * 1e-14
    R = V[:, keep] / np.sqrt(lam[keep])
    Praw = Kft @ R
    mu, Wv = np.linalg.eigh(Praw.T @ Praw)
    idx = np.argsort(mu)[::-1][:krank]
    return Praw @ Wv[:, idx]          # float64 [N, krank]


def _host_constants(image, mask):
    img64 = np.asarray(image, dtype=np.float64).reshape(H, W)
    m = np.asarray(mask).reshape(-1)
    f64 = img64.reshape(-1)

    P3 = _nystrom_P(f64).reshape(H, W, KRANK)
    b = np.where(m == 0, np.log(EPS), -np.log(EPS)).reshape(H, W)
    h0 = np.tanh(b / 2.0)

    # channel-major [y, (ch, x)], bilateral channels only
    praw = np.ascontiguousarray(np.transpose(P3, (0, 2, 1)))
    wp1 = np.ascontiguousarray(np.transpose(10.0 * P3, (0, 2, 1))
                               * h0[:, None, :])

    idx = np.arange(96, dtype=np.float64)
    d2 = (idx[:, None] - idx[None, :]) ** 2

    to32 = lambda a: np.ascontiguousarray(a, dtype=np.float32)
    return {
        "s1": to32(np.exp(-d2 / 5000.0)),
        "g1": to32(np.exp(-d2 / 18.0)),
        "h0": to32(h0),
        "cb": to32(b),
        "wp1": to32(wp1.reshape(H, KW)),
        "praw": to32(praw.reshape(H, KW)),
    }


# ------------------------- device program -------------------------

def _build():
    import concourse.bacc as bacc
    import concourse.mybir as mybir
    import concourse.tile as tile

    F32 = mybir.dt.float32
    AF = mybir.ActivationFunctionType
    ALU = mybir.AluOpType

    nc = bacc.Bacc("TRN2", target_bir_lowering=False, debug=False,
                   num_devices=NCORES)

    t_in = {}
    for name, shape in [("s1", [96, 96]), ("g1", [96, 96]),
                        ("h0", [96, 96]), ("cb", [96, 96]),
                        ("wp1", [96, KW]), ("praw", [96, KW])]:
        t_in[name] = nc.dram_tensor(name, shape, F32, kind="ExternalInput")
    out_t = nc.dram_tensor("logit_out", [96, 96], F32, kind="ExternalOutput")

    COFF = [0, 8, 16, 24]        # channel offset per bilateral chunk

    with tile.TileContext(nc) as tc:
        with (
            tc.tile_pool(name="const", bufs=1) as cpool,
            tc.tile_pool(name="work", bufs=2) as wpool,
            tc.tile_pool(name="psA", bufs=2, space="PSUM") as psA,
            tc.tile_pool(name="psM", bufs=2, space="PSUM") as psM,
        ):
            sb = {}
            for name in ["s1", "g1", "h0", "cb"]:
                sb[name] = cpool.tile([96, 96], F32, tag=name,
                                      name=f"sb_{name}")
                nc.sync.dma_start(sb[name][:], t_in[name][:])
            # big tensors stream in per chunk so iteration 1 starts as soon
            # as wp1 chunk 0 lands (not after the whole 1MB transfer).
            for name in ["wp1", "praw"]:
                sb[name] = cpool.tile([96, KW], F32, tag=name,
                                      name=f"sb_{name}")
                for c in range(NCHUNK):
                    w = CHUNKS[c] * 96
                    o = COFF[c] * 96
                    nc.sync.dma_start(sb[name][:, o:o + w],
                                      t_in[name][:, o:o + w])

            # p10 derived on device (saves 1MB of ramp DMA)
            p10 = cpool.tile([96, KW], F32, tag="p10")
            nc.vector.tensor_scalar_mul(p10[:], sb["praw"][:], 10.0)

            def rview(ap, r=8):
                return ap.rearrange("p (r x) -> p r x", r=r)

            def pview(ap, r=8):
                return ap.rearrange("p (r z) -> p r z", r=r)[:, :, 0:96]

            # pre-warm the PE HAM clock-gate while input DMAs stream
            for j in range(14):
                warm = psA.tile([96, 8 * 128], F32, tag="ptA",
                                name=f"prewarm{j}")
                nc.tensor.matmul(warm[:, 0:96], sb["s1"][:], sb["s1"][:],
                                 start=True, stop=True)

            h = sb["h0"]
            for it in range(NITER):
                # ---- gaussian micro-chunk first: its stage A reads h
                # directly (no wp multiply), so the PE starts the moment
                # tanh lands while the DVE computes the wp chunks.
                ptAg = psA.tile([96, 8 * 128], F32, tag="ptA",
                                name=f"Ag{it}")
                nc.tensor.matmul(ptAg[:, 0:96], h[:], sb["g1"][:],
                                 start=True, stop=True)

                # bilateral wp multiplies (DVE, strict FIFO: all first)
                wpcs = []
                for c in range(NCHUNK):
                    w = CHUNKS[c] * 96
                    o = COFF[c] * 96
                    if it == 0:
                        wpcs.append(sb["wp1"][:, o:o + w])
                    else:
                        wpt = wpool.tile([96, w], F32, tag=f"wp{c}",
                                         name=f"wp{it}_{c}")
                        nc.vector.tensor_mul(
                            rview(wpt[:], CHUNKS[c]),
                            rview(p10[:, o:o + w], CHUNKS[c]),
                            h[:].unsqueeze(1).broadcast_to(
                                [96, CHUNKS[c], 96]))
                        wpcs.append(wpt[:])

                # base = cb - 13h (DVE, off the critical path)
                basev = wpool.tile([96, 96], F32, tag="base",
                                   name=f"base{it}")
                nc.vector.scalar_tensor_tensor(
                    basev[:], h[:], -13.0, sb["cb"][:],
                    op0=ALU.mult, op1=ALU.add)

                tsg = wpool.tile([96, 96], F32, tag="tsg", name=f"tsg{it}")
                nc.scalar.activation(tsg[:], ptAg[:, 0:96], AF.Copy)

                ptAs, tss, ptMs, mms = {}, {}, {}, {}

                def emit_A(c):
                    ptA = psA.tile([96, 8 * 128], F32, tag="ptA",
                                   name=f"A{it}_{c}")
                    for r in range(CHUNKS[c]):
                        nc.tensor.matmul(ptA[:, r * 128:r * 128 + 96],
                                         wpcs[c][:, r * 96:(r + 1) * 96],
                                         sb["s1"][:], start=True, stop=True)
                    ptAs[c] = ptA

                def emit_ts(c):
                    ts = wpool.tile([96, CHUNKS[c] * 96], F32, tag=f"ts{c}",
                                    name=f"ts{it}_{c}")
                    nc.scalar.activation(
                        rview(ts[:], CHUNKS[c]),
                        pview(ptAs[c][:, :CHUNKS[c] * 128], CHUNKS[c]),
                        AF.Copy)
                    tss[c] = ts

                def emit_B(c):
                    ptM = psM.tile([96, 8 * 128], F32, tag="ptM",
                                   name=f"M{it}_{c}")
                    for r in range(CHUNKS[c]):
                        nc.tensor.matmul(ptM[:, r * 128:r * 128 + 96],
                                         tss[c][:, r * 96:(r + 1) * 96],
                                         sb["s1"][:], start=True, stop=True)
                    ptMs[c] = ptM

                def emit_mm(c):
                    w = CHUNKS[c] * 96
                    o = COFF[c] * 96
                    mm = wpool.tile([96, w], F32, tag=f"mm{c}",
                                    name=f"mm{it}_{c}")
                    nc.vector.tensor_mul(
                        rview(mm[:], CHUNKS[c]),
                        pview(ptMs[c][:, :CHUNKS[c] * 128], CHUNKS[c]),
                        rview(sb["praw"][:, o:o + w], CHUNKS[c]))
                    mms[c] = mm

                # PE order: Ag A0 Bg A1 B0 A2 B1 A3 B2 B3 (+warm dummies)
                emit_A(0)
                ptMg = psM.tile([96, 8 * 128], F32, tag="ptM",
                                name=f"Mg{it}")
                nc.tensor.matmul(ptMg[:, 0:96], tsg[:], sb["g1"][:],
                                 start=True, stop=True)
                emit_ts(0)
                # gaussian message: mm_g = 3 * (G h G), combined with base
                # early so the tail only handles the last bilateral chunk
                mmg = wpool.tile([96, 96], F32, tag="mmg", name=f"mmg{it}")
                nc.vector.tensor_scalar_mul(mmg[:], ptMg[:, 0:96], 3.0)
                gb = wpool.tile([96, 96], F32, tag="gb", name=f"gb{it}")
                nc.vector.tensor_add(gb[:], mmg[:], basev[:])

                emit_A(1); emit_ts(1)
                emit_B(0); emit_mm(0)
                emit_A(2); emit_ts(2)
                emit_B(1); emit_mm(1)
                a01 = wpool.tile([96, CW], F32, tag="a01", name=f"a01_{it}")
                nc.vector.tensor_add(a01[:], mms[0][:], mms[1][:])
                emit_A(3); emit_ts(3)
                emit_B(2); emit_mm(2)
                a2 = wpool.tile([96, CW], F32, tag="a2", name=f"a2_{it}")
                nc.vector.tensor_add(a2[:], a01[:], mms[2][:])
                f1 = wpool.tile([96, 384], F32, tag="f1", name=f"f1_{it}")
                nc.vector.tensor_add(f1[:], a2[:, :384], a2[:, 384:])
                f2 = wpool.tile([96, 192], F32, tag="f2", name=f"f2_{it}")
                nc.vector.tensor_add(f2[:], f1[:, :192], f1[:, 192:])
                f3 = wpool.tile([96, 96], F32, tag="f3", name=f"f3_{it}")
                nc.vector.tensor_add(f3[:], f2[:, :96], f2[:, 96:])
                fb = wpool.tile([96, 96], F32, tag="fb", name=f"fb_{it}")
                nc.vector.tensor_add(fb[:], f3[:], gb[:])
                emit_B(3); emit_mm(3)

                # back-to-back dummy matmuls cover the short PE-idle tail
                # (HAM stays at full rate only under sustained activity)
                if it < NITER - 1:
                    for j in range(12):
                        warm = psA.tile([96, 8 * 128], F32, tag="ptA",
                                        name=f"warm{it}_{j}")
                        nc.tensor.matmul(warm[:, 0:96], sb["s1"][:],
                                         sb["s1"][:], start=True, stop=True)

                # tail: fold the last (4-channel) chunk and combine
                u1 = wpool.tile([96, 192], F32, tag="u1", name=f"u1_{it}")
                nc.vector.tensor_add(u1[:], mms[3][:, :192],
                                     mms[3][:, 192:])
                u2 = wpool.tile([96, 96], F32, tag="u2", name=f"u2_{it}")
                nc.vector.tensor_add(u2[:], u1[:, :96], u1[:, 96:])
                logit = wpool.tile([96, 96], F32, tag="logit",
                                   name=f"logit{it}")
                nc.vector.tensor_add(logit[:], u2[:], fb[:])

                if it == NITER - 1:
                    nc.sync.dma_start(out_t[:], logit[:])
                else:
                    h2 = cpool.tile([96, 96], F32, tag=f"h{it}",
                                    name=f"h{it}")
                    nc.scalar.activation(h2[:], logit[:], AF.Tanh, scale=0.5)
                    h = h2

    nc.compile()
    return nc


def _get_nc():
    if "nc" not in _CACHE:
        _CACHE["nc"] = _build()
    return _CACHE["nc"]


# ------------------------- entry point -------------------------

def kernel(image, mask):
    global LAST_RESULTS
    import os
    from concourse.bass_utils import run_bass_kernel_spmd

    shared = _host_constants(image, mask)
    nc = _get_nc()
    in_maps = [dict(shared) for _ in range(NCORES)]
    trace = bool(int(os.environ.get("KERNEL_TRACE", "0")))
    kw = {}
    if trace and os.environ.get("KERNEL_TRACE_ALL"):
        kw["trace_cores"] = list(range(NCORES))
        kw["stitch_traces"] = True
    try:
        res = run_bass_kernel_spmd(nc, in_maps, core_ids=list(range(NCORES)),
                                   trace=trace, **kw)
    except Exception:
        # one retry for transient device hiccups
        res = run_bass_kernel_spmd(nc, in_maps, core_ids=list(range(NCORES)),
                                   trace=trace, **kw)
    LAST_RESULTS = res
    logit_yx = res.results[0]["logit_out"]          # [y, x]
    pred = (logit_yx < 0).astype(np.float32).reshape(1, 1, H, W)
    return pred
